# revision 1
# baseline (speedup 1.0000x reference)
"""Graph-transformer block on 8 Trainium2 NeuronCores.

Sharding: each core takes a 512-row q-slice of the 4096 nodes across ALL 4
heads (adj slice [4, 512, 4096] = 33.5MB/core; total adj read once). No
cross-core communication: each core finishes its attention rows, runs the
FFN on its own node slice, and writes its [512, 256] output slice.

Per-core pipeline (all orientations chosen so adj streams from DRAM in
naturally contiguous tiles):
  prep:  hT, q^T/k^T (head pairs packed on partitions), v (natural), weights
  attn:  S tile [128q,512j] = q^T.T @ k^T on PE (bf16)
         mk = (S * 1/sqrt(256)) * adj   (one DVE scalar_tensor_tensor, bf16 out)
         P = exp(mk) on ACT, accum_out -> softmax denominator rowsums
         P^T blocks via SBUF->SBUF DMA xbar transpose (bf16)
         x^T[hd] [64, 512q] += v_blk.T @ P^T on PE (PSUM accumulation)
  fin:   emb^T = x^T * (1/denom) broadcast  -> [256, 512] bf16 (2 tensors)
  ffn:   p1^T = relu(W1.T @ emb^T + b1); p2 = p1 @ W2 + b2 (natural [q,f])
         row softmax over 256 features; DMA out fp32
"""
import sys
import numpy as np

sys.path.insert(0, "/opt/trn_rl_repo")
import ml_dtypes  # noqa: E402

IN = 256
H = 4
DH = 64
NCORES = 8
F1 = 512
DOUT = 256
SCALE = 1.0 / 16.0  # 1/sqrt(IN)

_cache = {}


def build(n_nodes=4096, qs=512):
    """Build the bass program. n_nodes = total nodes (j extent),
    qs = q rows per core. Returns nc."""
    key = (n_nodes, qs)
    if key in _cache:
        return _cache[key]

    from contextlib import ExitStack
    import concourse.tile as tile
    from concourse import mybir, bacc
    from concourse.alu_op_type import AluOpType

    fp32, bf16 = mybir.dt.float32, mybir.dt.bfloat16
    AF = mybir.ActivationFunctionType
    AX = mybir.AxisListType

    NJT = n_nodes // 512   # 512-wide j tiles
    NJB = n_nodes // 128   # 128-wide j blocks
    NQC = qs // 128        # 128-row q chunks

    nc = bacc.Bacc("TRN2", target_bir_lowering=False, debug=False,
                   enable_asserts=False)

    adj_d = nc.dram_tensor("adj_s", [H, qs, n_nodes], fp32, kind="ExternalInput").ap()
    hT_d = nc.dram_tensor("hT", [IN, n_nodes], fp32, kind="ExternalInput").ap()
    hTq_d = nc.dram_tensor("hTq", [IN, qs], fp32, kind="ExternalInput").ap()
    wqp_d = nc.dram_tensor("wqp", [IN, H * DH], fp32, kind="ExternalInput").ap()
    wkp_d = nc.dram_tensor("wkp", [IN, H * DH], fp32, kind="ExternalInput").ap()
    wvp_d = nc.dram_tensor("wvp", [IN, H * DH], fp32, kind="ExternalInput").ap()
    w1_d = nc.dram_tensor("w1", [IN, F1], bf16, kind="ExternalInput").ap()
    w2_d = nc.dram_tensor("w2", [F1, DOUT], bf16, kind="ExternalInput").ap()
    b1_d = nc.dram_tensor("b1", [128, F1 // 128], fp32, kind="ExternalInput").ap()
    b2_d = nc.dram_tensor("b2", [1, DOUT], fp32, kind="ExternalInput").ap()
    out_d = nc.dram_tensor("out", [qs, DOUT], fp32, kind="ExternalOutput").ap()

    with ExitStack() as ctx:
        tc = ctx.enter_context(tile.TileContext(nc))
        pc = ctx.enter_context(tc.tile_pool(name="const", bufs=1))
        pst = ctx.enter_context(tc.tile_pool(name="stp", bufs=3, space="PSUM"))
        pxt = ctx.enter_context(tc.tile_pool(name="xtp", bufs=1, space="PSUM"))
        pa = ctx.enter_context(tc.tile_pool(name="adjp", bufs=4))
        pm = ctx.enter_context(tc.tile_pool(name="mkp", bufs=3))
        ppt = ctx.enter_context(tc.tile_pool(name="ptp", bufs=3))
        pptT = ctx.enter_context(tc.tile_pool(name="ptTp", bufs=2))
        psm = ctx.enter_context(tc.tile_pool(name="smallp", bufs=2))

        # ---------------- constants / prep ----------------
        hT_sb = [pc.tile([128, n_nodes], fp32, tag=f"hT{dc}", name=f"hT{dc}") for dc in range(2)]
        for dc in range(2):
            nc.gpsimd.dma_start(out=hT_sb[dc][:], in_=hT_d[dc * 128:(dc + 1) * 128, :])
        hTq_sb = [pc.tile([128, qs], fp32, tag=f"hTq{dc}", name=f"hTq{dc}") for dc in range(2)]
        for dc in range(2):
            nc.gpsimd.dma_start(out=hTq_sb[dc][:], in_=hTq_d[dc * 128:(dc + 1) * 128, :])

        # weight packs: cols dc*256 + (head*64+f)
        wq_sb = pc.tile([128, 2 * H * DH], fp32, tag="wq")
        wk_sb = pc.tile([128, 2 * H * DH], fp32, tag="wk")
        wv_sb = pc.tile([128, 2 * H * DH], fp32, tag="wv")
        for sb, d in ((wq_sb, wqp_d), (wk_sb, wkp_d), (wv_sb, wvp_d)):
            for dc in range(2):
                nc.gpsimd.dma_start(out=sb[:, dc * 256:(dc + 1) * 256],
                                    in_=d[dc * 128:(dc + 1) * 128, :])
        w1_sb = [pc.tile([128, F1], bf16, tag=f"w1_{dc}", name=f"w1_{dc}") for dc in range(2)]
        for dc in range(2):
            nc.gpsimd.dma_start(out=w1_sb[dc][:], in_=w1_d[dc * 128:(dc + 1) * 128, :])
        w2_sb = pc.tile([128, 4 * DOUT], bf16, tag="w2")
        for fc in range(4):
            nc.gpsimd.dma_start(out=w2_sb[:, fc * DOUT:(fc + 1) * DOUT],
                                in_=w2_d[fc * 128:(fc + 1) * 128, :])
        b1_sb = pc.tile([128, F1 // 128], fp32, tag="b1")
        nc.gpsimd.dma_start(out=b1_sb[:], in_=b1_d[:, :])
        b2_sb = pc.tile([1, DOUT], fp32, tag="b2")
        nc.gpsimd.dma_start(out=b2_sb[:], in_=b2_d[:, :])
        b2_bc = pc.tile([128, DOUT], fp32, tag="b2_bc")
        nc.gpsimd.partition_broadcast(b2_bc[:], b2_sb[0:1, :])

        # q^T / k^T: head pairs packed on partitions (pair p -> heads 2p,2p+1)
        qT_sb = [pc.tile([128, qs], bf16, tag=f"qT{p}", name=f"qT{p}") for p in range(2)]
        for p in range(2):
            for qt in range(qs // 512):
                ps = pst.tile([128, 512], fp32, tag="st")
                for dc in range(2):
                    nc.tensor.matmul(ps[:],
                                     wq_sb[:, dc * 256 + p * 128: dc * 256 + (p + 1) * 128],
                                     hTq_sb[dc][:, qt * 512:(qt + 1) * 512],
                                     start=(dc == 0), stop=(dc == 1))
                nc.vector.tensor_copy(qT_sb[p][:, qt * 512:(qt + 1) * 512], ps[:])
        kT_sb = [pc.tile([128, n_nodes], bf16, tag=f"kT{p}", name=f"kT{p}") for p in range(2)]
        for p in range(2):
            for jt in range(NJT):
                ps = pst.tile([128, 512], fp32, tag="st")
                for dc in range(2):
                    nc.tensor.matmul(ps[:],
                                     wk_sb[:, dc * 256 + p * 128: dc * 256 + (p + 1) * 128],
                                     hT_sb[dc][:, jt * 512:(jt + 1) * 512],
                                     start=(dc == 0), stop=(dc == 1))
                nc.vector.tensor_copy(kT_sb[p][:, jt * 512:(jt + 1) * 512], ps[:])
        # v natural [128j, NJB*256] bf16, block jb cols jb*256 + head*64 + f
        v_sb = pc.tile([128, NJB * 256], bf16, tag="v")
        for jb in range(NJB):
            ps = pst.tile([128, 256], fp32, tag="st")
            for dc in range(2):
                nc.tensor.matmul(ps[:], hT_sb[dc][:, jb * 128:(jb + 1) * 128],
                                 wv_sb[:, dc * 256:(dc + 1) * 256],
                                 start=(dc == 0), stop=(dc == 1))
            nc.vector.tensor_copy(v_sb[:, jb * 256:(jb + 1) * 256], ps[:])

        # ---------------- attention ----------------
        embT_sb = [pc.tile([128, qs], bf16, tag=f"embT{p}", name=f"embT{p}") for p in range(2)]
        xt = [pxt.tile([64, qs], fp32, tag=f"xt{hd}", name=f"xt{hd}") for hd in range(H)]

        for hd in range(H):
            p, off = hd // 2, (hd % 2) * 64
            rs_hd = psm.tile([128, NQC * NJT], fp32, tag="rs")  # col qc*NJT+jt
            for jt in range(NJT):
                ptTs = [pptT.tile([128, qs], bf16, tag=f"ptT{jj}", name=f"ptT{jj}_{hd}_{jt}") for jj in range(4)]
                for qc in range(NQC):
                    aj = pa.tile([128, 512], fp32, tag="aj")
                    nc.gpsimd.dma_start(out=aj[:],
                                        in_=adj_d[hd, qc * 128:(qc + 1) * 128,
                                                  jt * 512:(jt + 1) * 512])
                    st = pst.tile([128, 512], fp32, tag="st")
                    nc.tensor.matmul(st[:],
                                     qT_sb[p][off:off + 64, qc * 128:(qc + 1) * 128],
                                     kT_sb[p][off:off + 64, jt * 512:(jt + 1) * 512],
                                     start=True, stop=True)
                    mk = pm.tile([128, 512], bf16, tag="mk")
                    nc.vector.scalar_tensor_tensor(mk[:], st[:], SCALE, aj[:],
                                                   AluOpType.mult, AluOpType.mult)
                    pt = ppt.tile([128, 512], bf16, tag="pt")
                    c = qc * NJT + jt
                    nc.scalar.activation(pt[:], mk[:], AF.Exp,
                                         accum_out=rs_hd[:, c:c + 1])
                    for jj in range(4):
                        nc.sync.dma_start(out=ptTs[jj][:, qc * 128:(qc + 1) * 128],
                                          in_=pt[:, jj * 128:(jj + 1) * 128],
                                          transpose=True)
                for jj in range(4):
                    jb = jt * 4 + jj
                    nc.tensor.matmul(xt[hd][:],
                                     v_sb[:, jb * 256 + hd * 64: jb * 256 + hd * 64 + 64],
                                     ptTs[jj][:],
                                     start=(jb == 0), stop=(jb == NJB - 1))
            # denominators -> reciprocal, transposed to [1, qs] via tiny DMAs
            recip_hd = psm.tile([128, NQC], fp32, tag="recip")
            recipT_hd = psm.tile([1, qs], fp32, tag="recipT")
            for qc in range(NQC):
                dsum = psm.tile([128, 1], fp32, tag="dsum")
                nc.vector.tensor_reduce(dsum[:], rs_hd[:, qc * NJT:(qc + 1) * NJT],
                                        axis=AX.X, op=AluOpType.add)
                nc.vector.reciprocal(recip_hd[:, qc:qc + 1], dsum[:])
                nc.sync.dma_start(out=recipT_hd[0:1, qc * 128:(qc + 1) * 128],
                                  in_=recip_hd[:, qc:qc + 1])
            rT_bc = psm.tile([64, qs], fp32, tag="rT_bc")
            nc.gpsimd.partition_broadcast(rT_bc[:], recipT_hd[0:1, :])
            nc.vector.tensor_tensor(embT_sb[p][off:off + 64, :], xt[hd][:],
                                    rT_bc[:], AluOpType.mult)

        # ---------------- FFN + row softmax ----------------
        # p1^T chunk fc occupies cols [fc*qs, (fc+1)*qs)
        p1_sb = pc.tile([128, (F1 // 128) * qs], bf16, tag="p1")
        for fc in range(F1 // 128):
            ps = pst.tile([128, qs], fp32, tag="st")
            for dc in range(2):
                nc.tensor.matmul(ps[:], w1_sb[dc][:, fc * 128:(fc + 1) * 128],
                                 embT_sb[dc][:], start=(dc == 0), stop=(dc == 1))
            nc.scalar.activation(p1_sb[:, fc * qs:(fc + 1) * qs], ps[:], AF.Relu,
                                 bias=b1_sb[:, fc:fc + 1])
        for qc in range(NQC):
            ps2 = pst.tile([128, DOUT], fp32, tag="st")
            for fc in range(F1 // 128):
                nc.tensor.matmul(ps2[:],
                                 p1_sb[:, fc * qs + qc * 128: fc * qs + (qc + 1) * 128],
                                 w2_sb[:, fc * DOUT:(fc + 1) * DOUT],
                                 start=(fc == 0), stop=(fc == F1 // 128 - 1))
            t = psm.tile([128, DOUT], fp32, tag="t")
            nc.vector.tensor_tensor(t[:], ps2[:], b2_bc[:], AluOpType.add)
            mx = psm.tile([128, 1], fp32, tag="mx")
            nc.vector.tensor_reduce(mx[:], t[:], axis=AX.X, op=AluOpType.max,
                                    negate=True)
            e = psm.tile([128, DOUT], fp32, tag="e")
            nc.scalar.activation(e[:], t[:], AF.Exp, bias=mx[:])
            sm = psm.tile([128, 1], fp32, tag="sm")
            nc.vector.tensor_reduce(sm[:], e[:], axis=AX.X, op=AluOpType.add)
            rc = psm.tile([128, 1], fp32, tag="rc")
            nc.vector.reciprocal(rc[:], sm[:])
            o = psm.tile([128, DOUT], fp32, tag="o")
            nc.vector.tensor_scalar_mul(o[:], e[:], rc[:])
            nc.sync.dma_start(out=out_d[qc * 128:(qc + 1) * 128, :], in_=o[:])

    nc.compile()
    _cache[key] = nc
    return nc


def make_in_maps(h, adj, Wq, Wk, Wv, W1, b1, W2, b2, n_nodes, qs, ncores):
    h = np.asarray(h, np.float32)
    adj = np.asarray(adj, np.float32)
    hT = np.ascontiguousarray(h.T)
    WqP = np.ascontiguousarray(np.asarray(Wq, np.float32).transpose(1, 0, 2).reshape(IN, H * DH))
    WkP = np.ascontiguousarray(np.asarray(Wk, np.float32).transpose(1, 0, 2).reshape(IN, H * DH))
    WvP = np.ascontiguousarray(np.asarray(Wv, np.float32).transpose(1, 0, 2).reshape(IN, H * DH))
    W1b = np.asarray(W1, np.float32).astype(ml_dtypes.bfloat16)
    W2b = np.asarray(W2, np.float32).astype(ml_dtypes.bfloat16)
    b1r = np.ascontiguousarray(np.asarray(b1, np.float32).reshape(F1 // 128, 128).T)
    b2r = np.asarray(b2, np.float32).reshape(1, DOUT)
    in_maps = []
    for c in range(ncores):
        q0 = c * qs
        in_maps.append({
            "adj_s": np.ascontiguousarray(adj[:, q0:q0 + qs, :]),
            "hT": hT,
            "hTq": np.ascontiguousarray(hT[:, q0:q0 + qs]),
            "wqp": WqP, "wkp": WkP, "wvp": WvP,
            "w1": W1b, "w2": W2b, "b1": b1r, "b2": b2r,
        })
    return in_maps


def kernel(h, adj, Wq, Wk, Wv, W1, b1, W2, b2):
    import os
    n_nodes, qs = 4096, 512
    nc = build(n_nodes, qs)
    from concourse.bass_utils import run_bass_kernel_spmd
    in_maps = make_in_maps(h, adj, Wq, Wk, Wv, W1, b1, W2, b2, n_nodes, qs, NCORES)
    trace = bool(os.environ.get("BASS_KERNEL_TRACE"))
    res = run_bass_kernel_spmd(nc, in_maps, list(range(NCORES)), trace=trace)
    if trace and res.exec_time_ns is not None:
        print(f"HW exec time: {res.exec_time_ns} ns")
        kernel.last_exec_time_ns = res.exec_time_ns
    out = np.concatenate([np.asarray(res.results[c]["out"]) for c in range(NCORES)],
                         axis=0)
    return out.astype(np.float32)



# revision 7
# speedup vs baseline: 9.4041x; 9.4041x over previous
"""Graph-transformer block on 8 Trainium2 NeuronCores.

Sharding: each core takes a 512-row q-slice of the 4096 nodes across ALL 4
heads. No cross-core communication: each core computes attention for its
q rows, runs the FFN on its node slice, writes its [512, 256] output slice.

Per-core pipeline (transposed-S orientation — no on-device transposes):
  prep:  hT bf16; qT/kT per head pair packed on partitions; v with an
         interleaved all-ones column per head (the softmax denominator
         rides the PV matmul as output row 64). PSUM->SBUF prep copies go
         through the Scalar engine (ACT) which has slack; GPSIMD cannot
         read PSUM at all.
  attn:  per 128-wide j-block jb:
           one fp8 DMA of adjT for all 4 heads [128j, 4*512q]
           per head pair g: two S^T matmuls into one [128, 2, 512] PSUM
             tile (head 2g+i from partitions i*64 of the packed kT/qT)
           mk = (S^T * 1/16) * adjT   (DVE STT, PSUM fp32 * fp8 -> bf16)
           P^T = exp(mk) on ACT, one [128, 2048] activation per j-block
           xaug[hd] [65, 512] += v_aug_blk.T @ P^T on PE (PSUM accum)
         PV matmuls lag the S^T stage by PIPE j-blocks so the in-order PE
         stream never waits on the DVE->ACT chain.
  fin:   embT = xaug[0:64] * recip(xaug[64]) broadcast -> [256, 512] bf16
  ffn:   p1^T = relu(W1.T @ embT + b1); p2 = p1 @ W2 + b2 (natural [q,f])
         row softmax over 256 features; DMA out fp32
"""
import sys
import numpy as np

sys.path.insert(0, "/opt/trn_rl_repo")
import ml_dtypes  # noqa: E402

IN = 256
H = 4
DH = 64
NCORES = 8
F1 = 512
DOUT = 256
N_NODES = 4096
QS = 512
NJB = N_NODES // 128
SCALE = 1.0 / 16.0  # 1/sqrt(IN)
PIPE = 2            # PV matmuls lag S^T by this many j-blocks

_cache = {}


def build():
    if "nc" in _cache:
        return _cache["nc"]

    from contextlib import ExitStack
    import concourse.tile as tile
    from concourse import mybir, bacc
    from concourse.alu_op_type import AluOpType

    fp32, bf16 = mybir.dt.float32, mybir.dt.bfloat16
    fp8 = mybir.dt.float8e4
    AF = mybir.ActivationFunctionType
    AX = mybir.AxisListType
    MUL = AluOpType.mult
    ADD = AluOpType.add

    nc = bacc.Bacc("TRN2", target_bir_lowering=False, debug=False,
                   enable_asserts=False)

    adjt_d = nc.dram_tensor("adjt", [N_NODES, H * QS], fp8, kind="ExternalInput").ap()
    hT_d = nc.dram_tensor("hT", [IN, N_NODES], bf16, kind="ExternalInput").ap()
    hTq_d = nc.dram_tensor("hTq", [IN, QS], bf16, kind="ExternalInput").ap()
    wqp_d = nc.dram_tensor("wqp", [IN, H * DH], bf16, kind="ExternalInput").ap()
    wkp_d = nc.dram_tensor("wkp", [IN, H * DH], bf16, kind="ExternalInput").ap()
    wvp_d = nc.dram_tensor("wvp", [IN, H * DH], bf16, kind="ExternalInput").ap()
    w1_d = nc.dram_tensor("w1", [IN, F1], bf16, kind="ExternalInput").ap()
    w2_d = nc.dram_tensor("w2", [F1, DOUT], bf16, kind="ExternalInput").ap()
    b1_d = nc.dram_tensor("b1", [128, F1 // 128], fp32, kind="ExternalInput").ap()
    b2_d = nc.dram_tensor("b2", [1, DOUT], fp32, kind="ExternalInput").ap()
    out_d = nc.dram_tensor("out", [QS, DOUT], fp32, kind="ExternalOutput").ap()

    with ExitStack() as ctx:
        tc = ctx.enter_context(tile.TileContext(nc))
        pc = ctx.enter_context(tc.tile_pool(name="const", bufs=1))
        pst = ctx.enter_context(tc.tile_pool(name="stp", bufs=2, space="PSUM"))
        pxt = ctx.enter_context(tc.tile_pool(name="xtp", bufs=1, space="PSUM"))
        pa = ctx.enter_context(tc.tile_pool(name="adjp", bufs=3))
        pm = ctx.enter_context(tc.tile_pool(name="mkp", bufs=3))
        ppt = ctx.enter_context(tc.tile_pool(name="ptp", bufs=3))
        psm = ctx.enter_context(tc.tile_pool(name="smallp", bufs=2))

        # ---------------- constants / prep ----------------
        hT_sb = [pc.tile([128, N_NODES], bf16, tag=f"hT{dc}", name=f"hT{dc}") for dc in range(2)]
        for dc in range(2):
            nc.gpsimd.dma_start(out=hT_sb[dc][:], in_=hT_d[dc * 128:(dc + 1) * 128, :])
        hTq_sb = [pc.tile([128, QS], bf16, tag=f"hTq{dc}", name=f"hTq{dc}") for dc in range(2)]
        for dc in range(2):
            nc.gpsimd.dma_start(out=hTq_sb[dc][:], in_=hTq_d[dc * 128:(dc + 1) * 128, :])

        # weight packs: cols dc*256 + (head*64+f)
        wq_sb = pc.tile([128, 2 * H * DH], bf16, tag="wq")
        wk_sb = pc.tile([128, 2 * H * DH], bf16, tag="wk")
        wv_sb = pc.tile([128, 2 * H * DH], bf16, tag="wv")
        for sb, d in ((wq_sb, wqp_d), (wk_sb, wkp_d), (wv_sb, wvp_d)):
            for dc in range(2):
                nc.gpsimd.dma_start(out=sb[:, dc * 256:(dc + 1) * 256],
                                    in_=d[dc * 128:(dc + 1) * 128, :])
        w1_sb = [pc.tile([128, F1], bf16, tag=f"w1_{dc}", name=f"w1_{dc}") for dc in range(2)]
        for dc in range(2):
            nc.gpsimd.dma_start(out=w1_sb[dc][:], in_=w1_d[dc * 128:(dc + 1) * 128, :])
        w2_sb = pc.tile([128, 4 * DOUT], bf16, tag="w2")
        for fc in range(4):
            nc.gpsimd.dma_start(out=w2_sb[:, fc * DOUT:(fc + 1) * DOUT],
                                in_=w2_d[fc * 128:(fc + 1) * 128, :])
        b1_sb = pc.tile([128, F1 // 128], fp32, tag="b1")
        nc.gpsimd.dma_start(out=b1_sb[:], in_=b1_d[:, :])
        b2_sb = pc.tile([1, DOUT], fp32, tag="b2")
        nc.gpsimd.dma_start(out=b2_sb[:], in_=b2_d[:, :])
        b2_bc = pc.tile([128, DOUT], fp32, tag="b2_bc")
        nc.gpsimd.partition_broadcast(b2_bc[:], b2_sb[0:1, :])

        # q^T / k^T: head pairs packed on partitions (pair p -> heads 2p,2p+1)
        qT_sb = [pc.tile([128, QS], bf16, tag=f"qT{p}", name=f"qT{p}") for p in range(2)]
        for p in range(2):
            ps = pst.tile([128, QS], fp32, tag="st")
            for dc in range(2):
                nc.tensor.matmul(ps[:],
                                 wq_sb[:, dc * 256 + p * 128: dc * 256 + (p + 1) * 128],
                                 hTq_sb[dc][:],
                                 start=(dc == 0), stop=(dc == 1))
            nc.scalar.copy(qT_sb[p][:], ps[:])
        kT_sb = [pc.tile([128, N_NODES], bf16, tag=f"kT{p}", name=f"kT{p}") for p in range(2)]
        for p in range(2):
            for jt in range(N_NODES // 1024):
                ps = pst.tile([128, 2, 512], fp32, tag="st")
                for half in range(2):
                    for dc in range(2):
                        nc.tensor.matmul(
                            ps[:, half, :],
                            wk_sb[:, dc * 256 + p * 128: dc * 256 + (p + 1) * 128],
                            hT_sb[dc][:, jt * 1024 + half * 512: jt * 1024 + (half + 1) * 512],
                            start=(dc == 0), stop=(dc == 1))
                nc.scalar.copy(kT_sb[p][:, jt * 1024:(jt + 1) * 1024],
                               ps[:, :, :])
        # v with interleaved ones columns: vp[:, jb*4+hd, 0:64] = v values for
        # head hd at j-block jb; vp[:, *, 64] = 1.0 (softmax denominator row)
        vp = pc.tile([128, NJB * H, DH + 1], bf16, tag="vp")
        nc.vector.memset(vp[:, :, DH:DH + 1], 1.0)
        for jq in range(NJB // 4):  # 4 j-blocks of v per psum tile
            ps = pst.tile([128, 16, DH], fp32, tag="st")
            for jj in range(4):
                jb = jq * 4 + jj
                for dc in range(2):
                    nc.tensor.matmul(ps[:, jj * 4:(jj + 1) * 4, :],
                                     hT_sb[dc][:, jb * 128:(jb + 1) * 128],
                                     wv_sb[:, dc * 256:(dc + 1) * 256],
                                     start=(dc == 0), stop=(dc == 1))
            nc.scalar.copy(vp[:, jq * 16:(jq + 1) * 16, 0:DH], ps[:, :, :])

        # ---------------- attention ----------------
        embT_sb = [pc.tile([128, QS], bf16, tag=f"embT{p}", name=f"embT{p}") for p in range(2)]
        xaug = [pxt.tile([DH + 1, QS], fp32, tag=f"xt{hd}", name=f"xt{hd}") for hd in range(H)]

        pt_q = []
        for jb in range(NJB + PIPE):
            if jb < NJB:
                aj = pa.tile([128, H * QS], fp8, tag="aj")
                nc.sync.dma_start(out=aj[:],
                                  in_=adjt_d[jb * 128:(jb + 1) * 128, :])
                mk4 = pm.tile([128, H, QS], bf16, tag="mk")
                pt4 = ppt.tile([128, H, QS], bf16, tag="pt")
                for g in range(2):  # head pair
                    st2 = pst.tile([128, 2, 512], fp32, tag="st")
                    for i in range(2):  # head 2g+i from partitions i*64
                        nc.tensor.matmul(
                            st2[:, i, :],
                            kT_sb[g][i * 64:(i + 1) * 64, jb * 128:(jb + 1) * 128],
                            qT_sb[g][i * 64:(i + 1) * 64, :],
                            start=True, stop=True)
                    nc.vector.scalar_tensor_tensor(
                        mk4[:, 2 * g:2 * g + 2, :], st2[:, :, :], SCALE,
                        aj[:, g * 1024:(g + 1) * 1024], MUL, MUL)
                nc.scalar.activation(pt4[:, :, :], mk4[:, :, :], AF.Exp)
                pt_q.append((jb, pt4))
            if jb >= PIPE:
                j2, pt = pt_q.pop(0)
                for hd in range(H):
                    nc.tensor.matmul(xaug[hd][:],
                                     vp[:, j2 * H + hd, 0:DH + 1],
                                     pt[:, hd, :],
                                     start=(j2 == 0), stop=(j2 == NJB - 1))

        for hd in range(H):
            p, off = hd // 2, (hd % 2) * 64
            rcp = psm.tile([1, QS], fp32, tag="rcp")
            nc.vector.reciprocal(rcp[:], xaug[hd][DH:DH + 1, :])
            rbc = psm.tile([128, QS], fp32, tag="rbc")
            nc.gpsimd.partition_broadcast(rbc[:], rcp[0:1, :])
            nc.vector.tensor_tensor(embT_sb[p][off:off + 64, :],
                                    xaug[hd][0:DH, :], rbc[off:off + 64, :], MUL)

        # ---------------- FFN + row softmax ----------------
        p1_sb = pc.tile([128, F1 // 128, QS], bf16, tag="p1")
        for fc in range(F1 // 128):
            ps = pst.tile([128, QS], fp32, tag="st")
            for dc in range(2):
                nc.tensor.matmul(ps[:], w1_sb[dc][:, fc * 128:(fc + 1) * 128],
                                 embT_sb[dc][:], start=(dc == 0), stop=(dc == 1))
            nc.scalar.activation(p1_sb[:, fc, :], ps[:], AF.Relu,
                                 bias=b1_sb[:, fc:fc + 1])
        for qc in range(QS // 128):
            ps2 = pst.tile([128, DOUT], fp32, tag="st")
            for fc in range(F1 // 128):
                nc.tensor.matmul(ps2[:],
                                 p1_sb[:, fc, qc * 128:(qc + 1) * 128],
                                 w2_sb[:, fc * DOUT:(fc + 1) * DOUT],
                                 start=(fc == 0), stop=(fc == F1 // 128 - 1))
            t = psm.tile([128, DOUT], fp32, tag="t")
            nc.vector.tensor_tensor(t[:], ps2[:], b2_bc[:], ADD)
            mx = psm.tile([128, 1], fp32, tag="mx")
            nc.vector.tensor_reduce(mx[:], t[:], axis=AX.X, op=AluOpType.max,
                                    negate=True)
            e = psm.tile([128, DOUT], fp32, tag="e")
            sm = psm.tile([128, 1], fp32, tag="sm")
            nc.scalar.activation(e[:], t[:], AF.Exp, bias=mx[:], accum_out=sm[:])
            rc = psm.tile([128, 1], fp32, tag="rc")
            nc.vector.reciprocal(rc[:], sm[:])
            o = psm.tile([128, DOUT], fp32, tag="o")
            nc.vector.tensor_scalar_mul(o[:], e[:], rc[:])
            nc.sync.dma_start(out=out_d[qc * 128:(qc + 1) * 128, :], in_=o[:])

    nc.compile()
    _cache["nc"] = nc
    return nc


def make_in_maps(h, adj, Wq, Wk, Wv, W1, b1, W2, b2):
    bf16 = ml_dtypes.bfloat16
    fp8 = ml_dtypes.float8_e4m3
    h32 = np.asarray(h, np.float32)
    hT = np.ascontiguousarray(h32.T).astype(bf16)
    WqP = np.asarray(Wq, np.float32).transpose(1, 0, 2).reshape(IN, H * DH).astype(bf16)
    WkP = np.asarray(Wk, np.float32).transpose(1, 0, 2).reshape(IN, H * DH).astype(bf16)
    WvP = np.asarray(Wv, np.float32).transpose(1, 0, 2).reshape(IN, H * DH).astype(bf16)
    W1b = np.asarray(W1, np.float32).astype(bf16)
    W2b = np.asarray(W2, np.float32).astype(bf16)
    b1r = np.ascontiguousarray(np.asarray(b1, np.float32).reshape(F1 // 128, 128).T)
    b2r = np.asarray(b2, np.float32).reshape(1, DOUT)
    # adj [H, N, N] fp32 0/1 -> fp8 (exact), then per-core transposed slice
    adj8 = np.asarray(adj, np.float32).astype(fp8)
    # adjT8 [j, hd, q_global]
    adjT8 = np.ascontiguousarray(adj8.transpose(2, 0, 1))
    in_maps = []
    for c in range(NCORES):
        q0 = c * QS
        adjt = np.ascontiguousarray(
            adjT8[:, :, q0:q0 + QS]).reshape(N_NODES, H * QS)
        in_maps.append({
            "adjt": adjt,
            "hT": hT,
            "hTq": np.ascontiguousarray(hT[:, q0:q0 + QS]),
            "wqp": WqP, "wkp": WkP, "wvp": WvP,
            "w1": W1b, "w2": W2b, "b1": b1r, "b2": b2r,
        })
    return in_maps


def kernel(h, adj, Wq, Wk, Wv, W1, b1, W2, b2):
    import os
    nc = build()
    from concourse.bass_utils import run_bass_kernel_spmd
    in_maps = make_in_maps(h, adj, Wq, Wk, Wv, W1, b1, W2, b2)
    trace = bool(os.environ.get("BASS_KERNEL_TRACE"))
    res = run_bass_kernel_spmd(nc, in_maps, list(range(NCORES)), trace=trace)
    if trace and res.exec_time_ns is not None:
        print(f"HW exec time: {res.exec_time_ns} ns")
        kernel.last_exec_time_ns = res.exec_time_ns
    out = np.concatenate([np.asarray(res.results[c]["out"]) for c in range(NCORES)],
                         axis=0)
    return out.astype(np.float32)
